# revision 8
# baseline (speedup 1.0000x reference)
"""Trainium2 Bass kernel: GNN message-passing layer (nn_GNNlayer).

Computes, for full inputs (A [N,N], x [N,DIN], theta [K], W [DOUT,DIN], b, k):
    S1 = D^-1/2 A D^-1/2           (D = diag(rowsum A))
    P  = I + t0*S1 + t1*S1^2       (t = sigmoid(theta))
    S2 = D2^-1/2 P D2^-1/2         (D2 = diag(rowsum P))
    M  = top-k mask per row of S2
    out = (S2*M) @ x @ W.T + b

Sharding: rows are split across 8 NeuronCores (512 rows each). Each core
streams the full A as the moving matmul operand, computes its row block of
everything, and two tiny AllGathers exchange the degree vectors d and d2.

Key algebra used on-device (per core, rows R):
    B = A_R @ diag(1/d) @ A                      (the S1^2 numerator)
    C = t0*A_R + t1*B                            (P = I + Dh C Dh restricted to R)
    d2_i = 1 + dinv_i * sum_j C_ij * dinv_j
    ranking value T_ij = C_ij * dinv_j * dinv2b_j   (rank of off-diag S2 entries)
    row top-k = {diagonal} + top-(k-1) of T (diag excluded)
    out_i = [c_off_i * sum_t T_val_t * x[j_t] + c_diag_i * x[i]] @ W.T + b
      c_off_i  = dinv2b_i * dinv_i
      c_diag_i = dinv2b_i^2 * (1 + dinv_i^2 * C_ii)

The big matmul runs in float32r (fp22 mantissa, full PE rate).
"""

import os
import sys
import time
from contextlib import ExitStack

import numpy as np

sys.path.insert(0, "/opt/trn_rl_repo")

import concourse.bass as bass  # noqa: E402
import concourse.tile as tile  # noqa: E402
from concourse import bacc, bass_utils, mybir  # noqa: E402
from concourse.masks import make_identity  # noqa: E402

P = 128
NCORES = 8

f32 = mybir.dt.float32
f32r = mybir.dt.float32r
u32 = mybir.dt.uint32
i16 = mybir.dt.int16
OP = mybir.AluOpType
AF = mybir.ActivationFunctionType
AX = mybir.AxisListType

BIGNEG = 1.0e30

LAST_RUN_INFO = {}
_PROGRAM_CACHE = {}


def _emit(tc, io, N, NB, DIN, k):
    """Emit the per-core program. io: dict of dram APs."""
    STAGE = int(os.environ.get("K_STAGE", "9"))
    nc = tc.nc
    topn = k - 1              # off-diagonal picks per row
    NT = NB // P              # i-tiles (row tiles of this core's block)
    LC = N // P               # contraction chunks
    # phase width: j-columns processed per phase; NT i-tiles * JC banks <= 8
    PW = min(N, (8 // NT) * 512)
    PHASES = N // PW
    JC = PW // 512            # psum banks per i-tile per phase

    ctx = ExitStack()
    with ctx:
        cst = ctx.enter_context(tc.tile_pool(name="cst", bufs=1))
        cpool = ctx.enter_context(tc.tile_pool(name="cmat", bufs=1))
        dram = ctx.enter_context(tc.tile_pool(name="dram", bufs=1, space="DRAM"))

        # ---------------- constants / tiny loads ----------------
        ident = cst.tile([P, P], f32)
        make_identity(nc, ident[:])

        col_iota = cst.tile([P, N], i16)
        nc.gpsimd.iota(col_iota[:], pattern=[[1, N]], base=0, channel_multiplier=0)

        iota8 = cst.tile([P, 8], i16)
        nc.gpsimd.iota(iota8[:], pattern=[[1, 8]], base=0, channel_multiplier=0)
        iota8f = cst.tile([P, 8], f32)
        nc.vector.tensor_copy(out=iota8f[:], in_=iota8[:])
        iota8m = cst.tile([P, 8], f32)  # t - 99
        nc.vector.tensor_scalar_add(iota8m[:], iota8f[:], -99.0)

        th_b = cst.tile([P, 2], f32)
        nc.sync.dma_start(out=th_b[:], in_=io["theta"].broadcast_to([P, 2]))
        # sigmoid(theta) = 1 / (1 + exp(-theta)); DVE reciprocal for accuracy
        th_e = cst.tile([P, 2], f32)
        nc.scalar.activation(th_e[:], th_b[:], AF.Exp, scale=-1.0)
        nc.vector.tensor_scalar_add(th_e[:], th_e[:], 1.0)
        ts_sb = cst.tile([P, 2], f32)
        nc.vector.reciprocal(ts_sb[:], th_e[:])

        b_rep = cst.tile([P, DIN], f32)
        nc.sync.dma_start(out=b_rep[:], in_=io["bvec"].broadcast_to([P, DIN]))

        wt_sb = cst.tile([DIN, DIN], f32r)
        nc.sync.dma_start(out=wt_sb[:], in_=io["wt"])

        rowf_sb = cst.tile([P, NT], f32)
        nc.sync.dma_start(out=rowf_sb[:], in_=io["rowf"])
        rowu_sb = cst.tile([P, NT], u32)
        nc.sync.dma_start(out=rowu_sb[:], in_=io["rowu"])

        # ---------------- block load + degree d ----------------
        C = [cpool.tile([P, N], f32, tag=f"c{it}", name=f"C{it}") for it in range(NT)]
        dm_sb = cst.tile([P, NT], f32)
        for it in range(NT):
            half = N // 2
            nc.sync.dma_start(out=C[it][:, 0:half],
                              in_=io["a_blk"][it * P:(it + 1) * P, 0:half])
            nc.sync.dma_start(out=C[it][:, half:N],
                              in_=io["a_blk"][it * P:(it + 1) * P, half:N])
            nc.vector.tensor_reduce(out=dm_sb[:, it:it + 1], in_=C[it][:],
                                    axis=AX.X, op=OP.add)
        # local per-row scale factors
        dinv2_blk = cst.tile([P, NT], f32)
        nc.vector.reciprocal(dinv2_blk[:], dm_sb[:])
        dinv_blk = cst.tile([P, NT], f32)
        nc.scalar.activation(dinv_blk[:], dinv2_blk[:], AF.Sqrt)
        # scale block in place by t0 (C := t0 * A_R)
        for it in range(NT):
            nc.vector.tensor_scalar_mul(C[it][:], C[it][:], ts_sb[:, 0:1])

        if STAGE < 2:
            return
        # ---------------- AllGather #1 (d) ----------------
        g1_in = dram.tile([NT, P], f32)
        g1_out = dram.tile([LC, P], f32)
        with tc.tile_pool(name="psA", bufs=2, space="PSUM") as psA:
            dmT_ps = psA.tile([NT, P], f32)
            nc.tensor.transpose(out=dmT_ps[:], in_=dm_sb[:], identity=ident[:])
            dmT = cst.tile([NT, P], f32)
            nc.scalar.activation(dmT[:], dmT_ps[:], AF.Copy)
            nc.sync.dma_start(out=g1_in[:], in_=dmT[:])
            nc.gpsimd.collective_compute(
                "AllGather", OP.bypass,
                replica_groups=[list(range(NCORES))],
                ins=[g1_in.opt()], outs=[g1_out.opt()],
            )
            da_sb = cst.tile([LC, P], f32)
            nc.sync.dma_start(out=da_sb[:], in_=g1_out[:])

            # dinv (flat, global order) to DRAM for broadcast loads
            rda = cst.tile([LC, P], f32)
            nc.vector.reciprocal(rda[:], da_sb[:])
            dinv32 = cst.tile([LC, P], f32)
            nc.scalar.activation(dinv32[:], rda[:], AF.Sqrt)
            dinv_flat = dram.tile([1, N], f32)
            nc.sync.dma_start(
                out=dinv_flat[:].rearrange("one (a b) -> (one a) b", a=LC),
                in_=dinv32[:])

            # W column scale = t1 / d  in [P, LC] layout (transpose of rda)
            wsc_ps = psA.tile([P, LC], f32)
            nc.tensor.transpose(out=wsc_ps[:], in_=rda[:], identity=ident[:LC, :LC])
            wscale = cst.tile([P, LC], f32)
            nc.scalar.activation(wscale[:], wsc_ps[:], AF.Copy, scale=ts_sb[:, 1:2])

        if STAGE < 3:
            return
        # ---------------- weights (A_R^T, scaled) ----------------
        mainctx = ExitStack()
        with mainctx:
            wpool = mainctx.enter_context(tc.tile_pool(name="wmat", bufs=1))
            wsb = wpool.tile([P, LC * NB], f32r)
            for lc in range(LC):
                sl = wsb[:, lc * NB:(lc + 1) * NB]
                nc.sync.dma_start(out=sl, in_=io["a_blkt"][lc * P:(lc + 1) * P, :])
                nc.scalar.activation(sl, sl.bitcast(f32), AF.Copy, scale=wscale[:, lc:lc + 1])

            if STAGE < 4:
                return
            # ---------------- main matmul loop ----------------
            astream = mainctx.enter_context(tc.tile_pool(name="astream", bufs=7))
            dsp = mainctx.enter_context(tc.tile_pool(name="dstripe", bufs=2))
            scrp = mainctx.enter_context(tc.tile_pool(name="scr", bufs=2))
            psM = mainctx.enter_context(tc.tile_pool(name="psM", bufs=8, space="PSUM"))

            d2part = [cst.tile([P, PHASES], f32, tag=f"d2p{it}", name=f"d2p{it}") for it in range(NT)]
            ciipart = [cst.tile([P, PHASES], f32, tag=f"cip{it}", name=f"cip{it}") for it in range(NT)]

            for q in range(PHASES):
                q0 = q * PW
                dstripe = dsp.tile([P, PW], f32, tag="dstripe")
                nc.sync.dma_start(out=dstripe[:],
                                  in_=dinv_flat[:, q0:q0 + PW].broadcast_to([P, PW]))
                psums = [psM.tile([P, 512], f32, tag="acc", name=f"ps{q}_{i}") for i in range(NT * JC)]
                for lc in range(LC):
                    asl = astream.tile([P, PW], f32r, tag="astream")
                    for h in range(JC):
                        nc.sync.dma_start(
                            out=asl[:, h * 512:(h + 1) * 512],
                            in_=io["a_full"][lc * P:(lc + 1) * P,
                                             q0 + h * 512:q0 + (h + 1) * 512])
                    SUB = os.environ.get("K_SUB", "full")
                    for it in range(NT if SUB != "dma" else 0):
                        lhs = wsb[:, lc * NB + it * P:lc * NB + (it + 1) * P]
                        for jc in range(JC):
                            nc.tensor.matmul(
                                out=psums[it * JC + jc][:],
                                lhsT=lhs,
                                rhs=asl[:, jc * 512:(jc + 1) * 512],
                                start=(lc == 0), stop=(lc == LC - 1))
                for it in range(NT if os.environ.get("K_SUB", "full") in ("full", "evac") else 0):
                    for jc in range(JC):
                        cs = C[it][:, q0 + jc * 512:q0 + (jc + 1) * 512]
                        nc.vector.tensor_add(out=cs, in0=psums[it * JC + jc][:], in1=cs)
                    if os.environ.get("K_SUB", "full") == "evac":
                        continue
                    stripe = C[it][:, q0:q0 + PW]
                    scr1 = scrp.tile([P, PW], f32, tag="scr")
                    nc.vector.tensor_mul(scr1[:], stripe, dstripe[:])
                    nc.vector.tensor_reduce(out=d2part[it][:, q:q + 1],
                                            in_=scr1[:], axis=AX.X, op=OP.add)
                    scr2 = scrp.tile([P, PW], f32, tag="scr")
                    nc.vector.tensor_scalar(
                        out=scr2[:], in0=col_iota[:, q0:q0 + PW],
                        scalar1=rowf_sb[:, it:it + 1], scalar2=None, op0=OP.is_equal)
                    scr3 = scrp.tile([P, PW], f32, tag="scr")
                    nc.vector.tensor_mul(scr3[:], stripe, scr2[:])
                    nc.vector.tensor_reduce(out=ciipart[it][:, q:q + 1],
                                            in_=scr3[:], axis=AX.X, op=OP.add)

        if STAGE < 5:
            return
        # ---------------- d2, AllGather #2, ranking scale ----------------
        tailctx = ExitStack()
        with tailctx:
            tp = tailctx.enter_context(tc.tile_pool(name="tail", bufs=1))
            tscr = tailctx.enter_context(tc.tile_pool(name="tscr", bufs=2))
            psT = tailctx.enter_context(tc.tile_pool(name="psT", bufs=4, space="PSUM"))

            d2m = tp.tile([P, NT], f32)
            cii = tp.tile([P, NT], f32)
            for it in range(NT):
                nc.vector.tensor_reduce(out=d2m[:, it:it + 1], in_=d2part[it][:],
                                        axis=AX.X, op=OP.add)
                nc.vector.tensor_reduce(out=cii[:, it:it + 1], in_=ciipart[it][:],
                                        axis=AX.X, op=OP.add)
            # d2 = 1 + dinv * sum
            nc.vector.tensor_mul(d2m[:], d2m[:], dinv_blk[:])
            nc.vector.tensor_scalar_add(d2m[:], d2m[:], 1.0)

            dinv2b2 = tp.tile([P, NT], f32)  # dinv2b^2 = 1/d2
            nc.vector.reciprocal(dinv2b2[:], d2m[:])
            dinv2b = tp.tile([P, NT], f32)
            nc.scalar.activation(dinv2b[:], dinv2b2[:], AF.Sqrt)

            g2_in = dram.tile([NT, P], f32)
            g2_out = dram.tile([LC, P], f32)
            d2T_ps = psT.tile([NT, P], f32, tag="tr")
            nc.tensor.transpose(out=d2T_ps[:], in_=d2m[:], identity=ident[:])
            d2T = tp.tile([NT, P], f32)
            nc.scalar.activation(d2T[:], d2T_ps[:], AF.Copy)
            nc.sync.dma_start(out=g2_in[:], in_=d2T[:])
            nc.gpsimd.collective_compute(
                "AllGather", OP.bypass,
                replica_groups=[list(range(NCORES))],
                ins=[g2_in.opt()], outs=[g2_out.opt()],
            )
            d2a_sb = tp.tile([LC, P], f32)
            nc.sync.dma_start(out=d2a_sb[:], in_=g2_out[:])

            # v = 1/sqrt(d * d2)  (global order), replicate to all partitions
            vtmp = tp.tile([LC, P], f32)
            nc.vector.tensor_mul(vtmp[:], da_sb[:], d2a_sb[:])
            nc.vector.reciprocal(vtmp[:], vtmp[:])
            v32 = tp.tile([LC, P], f32)
            nc.scalar.activation(v32[:], vtmp[:], AF.Sqrt)
            v_flat = dram.tile([1, N], f32)
            nc.sync.dma_start(
                out=v_flat[:].rearrange("one (a b) -> (one a) b", a=LC),
                in_=v32[:])
            v_rep = tp.tile([P, N], f32)
            nc.sync.dma_start(out=v_rep[:], in_=v_flat[:].broadcast_to([P, N]))

            # output coefficients
            c_off = tp.tile([P, NT], f32)
            nc.vector.tensor_mul(c_off[:], dinv2b[:], dinv_blk[:])
            c_diag = tp.tile([P, NT], f32)
            nc.vector.tensor_mul(c_diag[:], dinv2_blk[:], cii[:])
            nc.vector.tensor_scalar_add(c_diag[:], c_diag[:], 1.0)
            nc.vector.tensor_mul(c_diag[:], c_diag[:], dinv2b2[:])

            if STAGE < 6:
                return
            # ---------------- per-tile: rank, gather, combine ----------------
            for it in range(NT):
                T = tscr.tile([P, N], f32, tag="T")
                nc.vector.tensor_mul(T[:], C[it][:], v_rep[:])
                m8 = tscr.tile([P, 8], f32, tag="m8")
                nc.vector.max(out=m8[:], in_=T[:])
                i8 = tscr.tile([P, 8], u32, tag="i8")
                nc.vector.max_index(out=i8[:], in_max=m8[:], in_values=T[:])
                i8f = tscr.tile([P, 8], f32, tag="i8f")
                nc.vector.tensor_copy(out=i8f[:], in_=i8[:])
                kill = tscr.tile([P, 8], f32, tag="kill")
                nc.vector.tensor_scalar(out=kill[:], in0=i8f[:],
                                        scalar1=rowf_sb[:, it:it + 1],
                                        scalar2=None, op0=OP.is_equal)
                # delete diag entry from the candidate list
                m8k = tscr.tile([P, 8], f32, tag="m8k")
                nc.vector.tensor_scalar(out=m8k[:], in0=kill[:], scalar1=BIGNEG,
                                        scalar2=None, op0=OP.mult)
                nc.vector.tensor_sub(m8k[:], m8[:], m8k[:])
                # diag position among the 8 (99 if absent)
                posw = tscr.tile([P, 8], f32, tag="posw")
                pos = tscr.tile([P, 1], f32, tag="pos")
                nc.vector.tensor_mul(posw[:], kill[:], iota8m[:])
                nc.vector.tensor_reduce(out=pos[:], in_=posw[:], axis=AX.X, op=OP.add)
                nc.vector.tensor_scalar_add(pos[:], pos[:], 99.0)
                shift = tscr.tile([P, 8], f32, tag="shift")
                nc.vector.tensor_scalar(out=shift[:], in0=iota8f[:], scalar1=pos[:],
                                        scalar2=None, op0=OP.is_ge)
                # top-(k-1) values/indices skipping the diag slot
                val = tscr.tile([P, topn], f32, tag="val")
                nc.vector.tensor_sub(val[:], m8k[:, 1:1 + topn], m8k[:, 0:topn])
                nc.vector.tensor_mul(val[:], val[:], shift[:, 0:topn])
                nc.vector.tensor_add(val[:], val[:], m8k[:, 0:topn])
                idxf = tscr.tile([P, topn], f32, tag="idxf")
                nc.vector.tensor_sub(idxf[:], i8f[:, 1:1 + topn], i8f[:, 0:topn])
                nc.vector.tensor_mul(idxf[:], idxf[:], shift[:, 0:topn])
                nc.vector.tensor_add(idxf[:], idxf[:], i8f[:, 0:topn])
                idxu = tscr.tile([P, topn], u32, tag="idxu")
                nc.vector.tensor_copy(out=idxu[:], in_=idxf[:])

                xg = tscr.tile([P, (topn + 1) * DIN], f32, tag="xg")
                for t in range(topn):
                    nc.gpsimd.indirect_dma_start(
                        out=xg[:, t * DIN:(t + 1) * DIN], out_offset=None,
                        in_=io["x"],
                        in_offset=bass.IndirectOffsetOnAxis(ap=idxu[:, t:t + 1], axis=0))
                nc.gpsimd.indirect_dma_start(
                    out=xg[:, topn * DIN:(topn + 1) * DIN], out_offset=None,
                    in_=io["x"],
                    in_offset=bass.IndirectOffsetOnAxis(ap=rowu_sb[:, it:it + 1], axis=0))

                # z = c_diag * x[i] + c_off * sum_t val_t * x[j_t]
                cval = tscr.tile([P, topn], f32, tag="cval")
                nc.vector.tensor_scalar_mul(cval[:], val[:], c_off[:, it:it + 1])
                z = tscr.tile([P, DIN], f32, tag="z")
                nc.vector.tensor_scalar(
                    out=z[:], in0=xg[:, topn * DIN:(topn + 1) * DIN],
                    scalar1=c_diag[:, it:it + 1], scalar2=None, op0=OP.mult)
                zt = tscr.tile([P, DIN], f32, tag="zt")
                for t in range(topn):
                    nc.vector.tensor_scalar(
                        out=zt[:], in0=xg[:, t * DIN:(t + 1) * DIN],
                        scalar1=cval[:, t:t + 1], scalar2=None, op0=OP.mult)
                    nc.vector.tensor_add(z[:], z[:], zt[:])

                # out = z @ W.T + b
                zT_ps = psT.tile([DIN, P], f32, tag="tr")
                nc.tensor.transpose(out=zT_ps[:], in_=z[:], identity=ident[:])
                zT = tscr.tile([DIN, P], f32, tag="zT")
                nc.scalar.activation(zT[:].bitcast(f32r), zT_ps[:], AF.Copy)
                o_ps = psT.tile([P, DIN], f32, tag="ops")
                nc.tensor.matmul(out=o_ps[:], lhsT=zT[:].bitcast(f32r),
                                 rhs=wt_sb[:], start=True, stop=True)
                o_sb = tscr.tile([P, DIN], f32, tag="osb")
                nc.vector.tensor_add(o_sb[:], o_ps[:], b_rep[:])
                nc.sync.dma_start(out=io["out_blk"][it * P:(it + 1) * P, :], in_=o_sb[:])


def _build(N, NB, DIN, k):
    key = (N, NB, DIN, k, os.environ.get("K_STAGE", "9"), os.environ.get("K_SUB", "full"))
    if key in _PROGRAM_CACHE:
        return _PROGRAM_CACHE[key]
    NT = NB // P
    nc = bacc.Bacc("TRN2", target_bir_lowering=False, debug=False,
                   num_devices=NCORES)
    io = {
        "a_full": nc.dram_tensor("a_full", [N, N], f32r, kind="ExternalInput").ap(),
        "a_blk": nc.dram_tensor("a_blk", [NB, N], f32, kind="ExternalInput").ap(),
        "a_blkt": nc.dram_tensor("a_blkt", [N, NB], f32r, kind="ExternalInput").ap(),
        "x": nc.dram_tensor("x", [N, DIN], f32, kind="ExternalInput").ap(),
        "wt": nc.dram_tensor("wt", [DIN, DIN], f32r, kind="ExternalInput").ap(),
        "bvec": nc.dram_tensor("bvec", [1, DIN], f32, kind="ExternalInput").ap(),
        "theta": nc.dram_tensor("theta", [1, 2], f32, kind="ExternalInput").ap(),
        "rowf": nc.dram_tensor("rowf", [P, NT], f32, kind="ExternalInput").ap(),
        "rowu": nc.dram_tensor("rowu", [P, NT], u32, kind="ExternalInput").ap(),
        "out_blk": nc.dram_tensor("out_blk", [NB, DIN], f32, kind="ExternalOutput").ap(),
    }
    with tile.TileContext(nc) as tc:
        _emit(tc, io, N, NB, DIN, k)
    nc.compile()
    _PROGRAM_CACHE[key] = nc
    return nc


def make_in_maps(x, A, theta, W, b, k, N, NB, DIN):
    A = np.ascontiguousarray(np.asarray(A, np.float32))
    x = np.ascontiguousarray(np.asarray(x, np.float32))
    theta = np.ascontiguousarray(np.asarray(theta, np.float32)).reshape(1, 2)
    W = np.asarray(W, np.float32)
    b = np.ascontiguousarray(np.asarray(b, np.float32)).reshape(1, DIN)
    wt = np.ascontiguousarray(W.T)
    NT = NB // P
    in_maps = []
    for m in range(NCORES):
        rows = slice(m * NB, (m + 1) * NB)
        a_blk = np.ascontiguousarray(A[rows])
        a_blkt = np.ascontiguousarray(a_blk.T)
        ridx = (m * NB + np.arange(NB)).reshape(NT, P).T  # [P, NT]
        in_maps.append({
            "a_full": A,
            "a_blk": a_blk,
            "a_blkt": a_blkt,
            "x": x,
            "wt": wt,
            "bvec": b,
            "theta": theta,
            "rowf": np.ascontiguousarray(ridx.astype(np.float32)),
            "rowu": np.ascontiguousarray(ridx.astype(np.uint32)),
        })
    return in_maps


def kernel(x, A, theta, W, b, k, **extra):
    k = int(k)
    assert 1 <= k <= 8, f"k={k} unsupported"
    N = int(A.shape[0])
    DIN = int(x.shape[1])
    NB = N // NCORES
    nc = _build(N, NB, DIN, k)
    in_maps = make_in_maps(x, A, theta, W, b, k, N, NB, DIN)
    trace = bool(int(os.environ.get("BASS_KERNEL_TRACE", "0")))
    t0 = time.monotonic()
    res = bass_utils.run_bass_kernel_spmd(
        nc, in_maps, core_ids=list(range(NCORES)), trace=trace)
    t1 = time.monotonic()
    LAST_RUN_INFO.clear()
    LAST_RUN_INFO.update({
        "wall_s": t1 - t0,
        "exec_time_ns": res.exec_time_ns,
        "profile_json": res.profile_json,
    })
    out = np.concatenate([res.results[m]["out_blk"] for m in range(NCORES)], axis=0)
    return out.astype(np.float32)
